# revision 3
# baseline (speedup 1.0000x reference)
"""DCNv2 deformable RoI pooling on 8 Trainium2 NeuronCores — gather version.

Strategy (roi-sharded, dma_gather + matmul reduce, bf16 data path):
  - Host: replicate the reference's f32 sampling math from (rois, offset),
    derive for each roi a tight rectangular window and a dense weight matrix
    folding bilinear weights, validity and 1/count:
        out[bin, ch] = sum_px Wmat[px, bin] * Fwin[px, ch].
  - Feature map is NHWC bf16: one pixel = 512B contiguous.  All window
    pixels for a core are fetched with a few big SWDGE dma_gather calls
    (one 512B descriptor per pixel, spread across all 16 SDMA engines),
    instead of ~58 per-chunk dma_starts whose issue cost dominated.
  - Rois grouped by image (gather src base is compile-time), padded to 8 per
    image group, sorted by window size, dealt round-robin to the 8 cores so
    slot s has identical chunk count nch_s on every core (SPMD).
  - Pixel lists per roi are padded to 128-multiples; gather lays pixel i at
    partition i%128, so every matmul is K=128: psum[49, 256] accumulated
    over a slot's chunks with lhsT = wmat chunk [128, 49] (small LDWEIGHTS),
    rhs = win chunk [128, 256].
  - Psum copied (f32->bf16) to a [49, S*256] staging tile; out DMA'd in
    4 pieces as slots finish.
  - Host: reassemble [128, 256, 7, 7] in f32.
"""
import sys

sys.path.insert(0, "/opt/trn_rl_repo")

import numpy as np
import ml_dtypes

bf16 = ml_dtypes.bfloat16

SPATIAL_SCALE = 0.0625
POOLED = 7
SAMPLE = 4
TRANS_STD = 0.1
B, C, H, W = 2, 256, 160, 160
N_ROIS = 128
NCORES = 8
P, S = POOLED, SAMPLE
NBINS = P * P
PX_PER_IMG = H * W  # 25600 rows of 512B in NHWC bf16; fits int16 indices

f32 = np.float32


# ----------------------------------------------------------------- host plan

def _sample_math(rois, offset):
    rois = rois.astype(f32)
    offset = offset.astype(f32)
    b = rois[:, 0].astype(np.int32)
    x1, y1, x2, y2 = rois[:, 1], rois[:, 2], rois[:, 3], rois[:, 4]
    rsw = (np.round(x1) * f32(SPATIAL_SCALE) - f32(0.5)).astype(f32)
    rsh = (np.round(y1) * f32(SPATIAL_SCALE) - f32(0.5)).astype(f32)
    rew = ((np.round(x2) + f32(1.0)) * f32(SPATIAL_SCALE) - f32(0.5)).astype(f32)
    reh = ((np.round(y2) + f32(1.0)) * f32(SPATIAL_SCALE) - f32(0.5)).astype(f32)
    rw = np.maximum(rew - rsw, f32(0.1))
    rh = np.maximum(reh - rsh, f32(0.1))
    bw, bh = (rw / P).astype(f32), (rh / P).astype(f32)
    sw, sh = (bw / S).astype(f32), (bh / S).astype(f32)
    tx = offset[:, 0] * f32(TRANS_STD)
    ty = offset[:, 1] * f32(TRANS_STD)
    pw_i = np.arange(P, dtype=f32)
    ph_i = np.arange(P, dtype=f32)
    wstart = (pw_i[None, None, :] * bw[:, None, None] + rsw[:, None, None]
              + tx * rw[:, None, None]).astype(f32)
    hstart = (ph_i[None, :, None] * bh[:, None, None] + rsh[:, None, None]
              + ty * rh[:, None, None]).astype(f32)
    iw = np.arange(S, dtype=f32)
    x = (wstart[..., None] + iw * sw[:, None, None, None]).astype(f32)
    y = (hstart[..., None] + iw * sh[:, None, None, None]).astype(f32)
    validx = (x >= -0.5) & (x <= W - 0.5)
    validy = (y >= -0.5) & (y <= H - 0.5)
    xc = np.clip(x, f32(0.0), f32(W - 1.0))
    yc = np.clip(y, f32(0.0), f32(H - 1.0))
    x0 = np.floor(xc); x1c = np.ceil(xc)
    y0 = np.floor(yc); y1c = np.ceil(yc)
    dx = (xc - x0).astype(f32)
    dy = (yc - y0).astype(f32)
    cnt = (validx.sum(-1) * validy.sum(-1)).astype(f32)
    denom = np.maximum(cnt, f32(1.0))
    return dict(b=b, validx=validx, validy=validy,
                x0=x0.astype(np.int32), x1=x1c.astype(np.int32),
                y0=y0.astype(np.int32), y1=y1c.astype(np.int32),
                dx=dx, dy=dy, denom=denom)


def _plan(rois, offset):
    sm = _sample_math(rois, offset)
    nroi = sm["b"].shape[0]
    xmin = np.zeros(nroi, np.int64); xmax = np.zeros(nroi, np.int64)
    ymin = np.zeros(nroi, np.int64); ymax = np.zeros(nroi, np.int64)
    vx, vy = sm["validx"], sm["validy"]
    for n in range(nroi):
        joint = (vx[n].any(-1) & vy[n].any(-1))
        if not joint.any():
            continue
        selx = vx[n] & joint[..., None]
        sely = vy[n] & joint[..., None]
        xmin[n] = sm["x0"][n][selx].min(); xmax[n] = sm["x1"][n][selx].max()
        ymin[n] = sm["y0"][n][sely].min(); ymax[n] = sm["y1"][n][sely].max()
    w_need = (xmax - xmin + 1).astype(np.int64)
    h_need = (ymax - ymin + 1).astype(np.int64)
    px = w_need * h_need

    # group rois by image; pad each group to a multiple of NCORES with
    # dummy entries (roi index -1, zero weights, 1-px window)
    slots_of_group = []
    for img in range(B):
        idxs = [n for n in range(nroi) if sm["b"][n] == img]
        idxs.sort(key=lambda n: -px[n])
        npad = (-len(idxs)) % NCORES
        idxs += [-1] * npad
        slots_of_group.append([idxs[i * NCORES:(i + 1) * NCORES]
                               for i in range(len(idxs) // NCORES)])
    S0 = len(slots_of_group[0])
    slot_rois = slots_of_group[0] + slots_of_group[1]  # [S][NCORES] roi or -1
    SLOTS = len(slot_rois)

    nch = []
    for grp in slot_rois:
        m = max((int(px[n]) for n in grp if n >= 0), default=1)
        nch.append(-(-m // 128))

    # per-roi weight matrix in tight-window pixel space: [h*w, 49]
    wmats = {}
    for n in range(nroi):
        h, w = int(h_need[n]), int(w_need[n])
        Ay = np.zeros((NBINS, h), f32)
        Bx = np.zeros((NBINS, w), f32)
        vxn = sm["validx"][n].reshape(NBINS, S)
        vyn = sm["validy"][n].reshape(NBINS, S)
        x0 = sm["x0"][n].reshape(NBINS, S) - xmin[n]
        x1c = sm["x1"][n].reshape(NBINS, S) - xmin[n]
        y0 = sm["y0"][n].reshape(NBINS, S) - ymin[n]
        y1c = sm["y1"][n].reshape(NBINS, S) - ymin[n]
        dx = sm["dx"][n].reshape(NBINS, S)
        dy = sm["dy"][n].reshape(NBINS, S)
        bins = np.repeat(np.arange(NBINS), S)
        np.add.at(Bx, (bins, np.clip(x0, 0, w - 1).ravel()), ((1 - dx) * vxn).ravel())
        np.add.at(Bx, (bins, np.clip(x1c, 0, w - 1).ravel()), (dx * vxn).ravel())
        np.add.at(Ay, (bins, np.clip(y0, 0, h - 1).ravel()), ((1 - dy) * vyn).ravel())
        np.add.at(Ay, (bins, np.clip(y1c, 0, h - 1).ravel()), (dy * vyn).ravel())
        Wpx = Ay[:, :, None] * Bx[:, None, :] / sm["denom"][n].reshape(NBINS, 1, 1)
        wmats[n] = Wpx.reshape(NBINS, h * w).T.astype(f32)  # [h*w, 49]

    return dict(sm=sm, slot_rois=slot_rois, S0=S0, SLOTS=SLOTS, nch=nch,
                xmin=xmin, ymin=ymin, w_need=w_need, h_need=h_need,
                wmats=wmats)


def _gather_groups(nch, S0, max_chunks=8):
    """Split slots into gather groups (contiguous, not crossing the image
    boundary at slot S0), each with at most max_chunks chunks."""
    groups = []  # list of (slot_start, slot_end)
    s = 0
    SLOTS = len(nch)
    while s < SLOTS:
        limit = S0 if s < S0 else SLOTS
        e = s
        acc = 0
        while e < limit and acc + nch[e] <= max_chunks:
            acc += nch[e]
            e += 1
        if e == s:  # single slot exceeding max_chunks
            e = s + 1
        groups.append((s, e))
        s = e
    return groups


# --------------------------------------------------------------- bass program

_PROGRAM_CACHE = {}


def _build_program(nch, S0):
    import concourse.bass as bass
    import concourse.bacc as bacc
    import concourse.mybir as mybir
    import concourse.tile as tile

    SLOTS = len(nch)
    T = sum(nch)
    WMCOLS = T * NBINS
    OCOLS = SLOTS * C
    IDXCOLS = T * 8  # T*128 idxs / 16 partitions

    coff = np.concatenate([[0], np.cumsum(nch)]).astype(int)  # chunk offset/slot
    groups = _gather_groups(nch, S0)

    nc = bacc.Bacc("TRN2", target_bir_lowering=False, debug=False,
                   num_devices=NCORES)
    feat = nc.declare_dram_parameter("feat", [B * H * W * C], mybir.dt.bfloat16,
                                     isOutput=False)
    wmat = nc.declare_dram_parameter("wmat", [128 * WMCOLS], mybir.dt.bfloat16,
                                     isOutput=False)
    idxs = nc.declare_dram_parameter("idxs", [128 * IDXCOLS], mybir.dt.int16,
                                     isOutput=False)
    out = nc.declare_dram_parameter("out", [NBINS * OCOLS], mybir.dt.bfloat16,
                                    isOutput=True)

    with tile.TileContext(nc) as tc:
        with (
            tc.tile_pool(name="small", bufs=1) as small,
            tc.tile_pool(name="winp", bufs=1) as winp,
            tc.tile_pool(name="psum", bufs=8, space="PSUM") as psump,
        ):
            idxt = small.tile([128, IDXCOLS], mybir.dt.int16)
            wm = small.tile([128, WMCOLS], mybir.dt.bfloat16)
            ostage = small.tile([NBINS, OCOLS], mybir.dt.bfloat16)

            # index table (tiny) first so gathers can start immediately
            isrc = bass.AP(idxs[:].tensor, 0, [[IDXCOLS, 128], [1, IDXCOLS]])
            nc.sync.dma_start(idxt[:, :], isrc)

            # wmat in two pieces on the two HWDGE queues
            half = (WMCOLS // 2 + 63) & ~63
            for gi, (c0, c1) in enumerate(((0, half), (half, WMCOLS))):
                src = bass.AP(wmat[:].tensor, c0, [[WMCOLS, 128], [1, c1 - c0]])
                dst = bass.AP(wm[:].tensor, wm[:].offset + c0,
                              [[WMCOLS, 128], [1, c1 - c0]])
                (nc.sync if gi == 0 else nc.scalar).dma_start(dst, src)

            # one dma_gather per group: pixel i of the group lands at
            # partition i%128, chunk column i//128 (512B per pixel)
            win_tiles = {}
            for g, (s0, s1) in enumerate(groups):
                gch = int(coff[s1] - coff[s0])
                n_idx = gch * 128
                wt = winp.tile([128, gch * C], mybir.dt.bfloat16, tag=f"win{g}")
                dst = bass.AP(wt[:].tensor, wt[:].offset,
                              [[gch * C, 128], [C, gch], [1, C]])
                img = 0 if s1 <= S0 else 1
                src = bass.AP(feat[:].tensor, img * PX_PER_IMG * C,
                              [[C, PX_PER_IMG], [1, C]])
                ioff = int(coff[s0]) * 8  # idx columns consumed so far
                nc.gpsimd.dma_gather(
                    dst, src, idxt[:, ioff:ioff + gch * 8],
                    num_idxs=n_idx, num_idxs_reg=n_idx, elem_size=C)
                win_tiles[g] = (wt, int(coff[s0]))

            # matmuls: psum[49, 256] per slot accumulated over its chunks
            out_pieces = 4
            piece = -(-SLOTS // out_pieces)
            g = 0
            for s in range(SLOTS):
                while s >= groups[g][1]:
                    g += 1
                wt, gbase = win_tiles[g]
                pt = psump.tile([NBINS, C], mybir.dt.float32, tag="pt")
                for k in range(nch[s]):
                    ch = int(coff[s]) + k
                    nc.tensor.matmul(
                        pt[:, :],
                        wm[:, ch * NBINS:(ch + 1) * NBINS],
                        wt[:, (ch - gbase) * C:(ch - gbase + 1) * C],
                        start=(k == 0), stop=(k == nch[s] - 1),
                    )
                nc.vector.tensor_copy(ostage[:, s * C:(s + 1) * C], pt[:, :])

                if s % piece == piece - 1 or s == SLOTS - 1:
                    a = (s // piece) * piece * C
                    bcol = (s + 1) * C
                    osrc = ostage[:, a:bcol]
                    odst = bass.AP(out[:].tensor, a, [[OCOLS, NBINS], [1, bcol - a]])
                    (nc.sync if (s // piece) % 2 == 0 else nc.scalar).dma_start(
                        odst, osrc)

    nc.compile()
    return nc


# -------------------------------------------------------------------- kernel

TRACE = False
SIM = False
LAST_RESULTS = None


def _pack_inputs(pl):
    """Per-core idxs / wmat arrays."""
    slot_rois, nch = pl["slot_rois"], pl["nch"]
    SLOTS = len(slot_rois)
    T = sum(nch)
    WMCOLS = T * NBINS
    coff = np.concatenate([[0], np.cumsum(nch)]).astype(int)

    idx_all = []
    wm_all = []
    for c in range(NCORES):
        idx = np.zeros(T * 128, np.int16)
        wmc = np.zeros((128, WMCOLS), f32)
        for s in range(SLOTS):
            n = slot_rois[s][c]
            o = int(coff[s]) * 128
            K = nch[s] * 128
            if n < 0:
                # dummy: point at pixel 0 of the image, zero weights
                continue
            h, w = int(pl["h_need"][n]), int(pl["w_need"][n])
            base = int(pl["ymin"][n]) * W + int(pl["xmin"][n])
            yy, xx = np.divmod(np.arange(h * w), w)
            pix = (pl["ymin"][n] + yy) * W + (pl["xmin"][n] + xx)
            ids = np.full(K, base, np.int64)
            ids[:h * w] = pix
            idx[o:o + K] = ids.astype(np.int16)
            wpx = np.zeros((K, NBINS), f32)
            wpx[:h * w] = pl["wmats"][n]
            wch = wpx.reshape(nch[s], 128, NBINS)
            for k in range(nch[s]):
                wmc[:, (int(coff[s]) + k) * NBINS:(int(coff[s]) + k + 1) * NBINS] = wch[k]
        # wrap indices: idx j -> partition j%16, col j//16; replicate 8x
        idx16 = idx.reshape(-1, 16).T  # [16, T*8]
        idx128 = np.tile(idx16, (8, 1))  # [128, T*8]
        idx_all.append(np.ascontiguousarray(idx128).reshape(-1))
        wm_all.append(np.ascontiguousarray(wmc.astype(bf16)).reshape(-1))
    return idx_all, wm_all


def kernel(input, rois, offset):
    from concourse.bass_utils import run_bass_kernel_spmd

    input = np.asarray(input, f32)
    rois = np.asarray(rois, f32)
    offset = np.asarray(offset, f32)

    pl = _plan(rois, offset)
    nch, S0, SLOTS = pl["nch"], pl["S0"], pl["SLOTS"]

    nhwc = np.ascontiguousarray(np.transpose(input, (0, 2, 3, 1)).astype(bf16))
    feat_flat = nhwc.reshape(-1)

    idx_all, wm_all = _pack_inputs(pl)
    in_maps = [{"feat": feat_flat, "wmat": wm_all[c], "idxs": idx_all[c]}
               for c in range(NCORES)]

    key = (S0, tuple(nch))
    if key not in _PROGRAM_CACHE:
        _PROGRAM_CACHE[key] = _build_program(nch, S0)
    nc = _PROGRAM_CACHE[key]

    if SIM:
        from concourse.bass_interp import CoreSim
        results = []
        for c in range(min(NCORES, SIM if isinstance(SIM, int) else 1)):
            sim = CoreSim(nc)
            for k, v in in_maps[c].items():
                sim.tensor(k)[:] = v.reshape(sim.tensor(k).shape)
            sim.simulate()
            results.append({"out": np.asarray(sim.tensor("out")).copy()})
        res = type("R", (), {"results": results, "exec_time_ns": None})()
    else:
        kwargs = {}
        if TRACE:
            kwargs = dict(trace=True, trace_cores=list(range(NCORES)))
        res = run_bass_kernel_spmd(nc, in_maps, list(range(NCORES)), **kwargs)
    global LAST_RESULTS
    LAST_RESULTS = res

    out_full = np.zeros((N_ROIS, C, NBINS), f32)
    for c in range(len(res.results)):
        o = res.results[c]["out"].reshape(NBINS, SLOTS, C).astype(f32)
        for s in range(SLOTS):
            n = pl["slot_rois"][s][c]
            if n >= 0:
                out_full[n] = o[:, s, :].T
    return out_full.reshape(N_ROIS, C, P, P)
